# revision 14
# baseline (speedup 1.0000x reference)
"""Bahdanau attention kernel for Trainium2 (Bass/Tile), 8-core data-parallel.

Problem shapes: B=32, Tx=1024, enc_hid=dec_hid=attn=1024.

v10: bf16 + nt-paired weight sharing + interleaved HAM warmup.
  - All big matmul operands bf16 (validated: ctx rel ~2.8e-3, tol 2e-2).
  - Host pre-tiles every tensor so each DMA is a contiguous 2D slab.
  - Energy groups pair the two 512-wide t-halves: each w_encT chunk is
    loaded once and serves the nt0/nt1 matmuls back to back (measured
    ~46ns weight-switch penalty per matmul otherwise), and consecutive
    matmuls alternate PSUM banks.
  - exp via ACT accum_out => softmax sum free; alpha row->column via
    tiny PE transposes (no DRAM bounce).
  - PE stream software-pipelined: the next example's first two energy
    groups are emitted before the current example's softmax tail.

Math (per example b):
  dec_proj = W_dec @ dec_hidden[b]                 [attn]
  energy^T[a, t] = tanh(sum_e W_enc[a,e] enc[b,t,e] + dec_proj[a] + W_b[a])
  scores[t] = sum_a v[a] energy^T[a, t]
  alpha = softmax(scores + (mask-1)*50)
  context[e] = sum_t alpha[t] enc[b,t,e]
"""

from contextlib import ExitStack

import numpy as np
import ml_dtypes

import concourse.bass as bass
import concourse.tile as tile
from concourse import bacc, mybir
from concourse.masks import make_identity

F32 = mybir.dt.float32
BF16 = mybir.dt.bfloat16
AF = mybir.ActivationFunctionType
BF = ml_dtypes.bfloat16

P = 128
N_CORES = 8
B_LOC = 4            # examples per core
TX = 1024
E = 1024             # enc_hid
A = 1024             # attn
D = 1024             # dec_hid
EO = E // P
AO = A // P
TO = TX // P
DO = D // P
NT = 2               # 512-wide t-halves
ET = 2               # 512-wide e-halves


def build_nc():
    nc = bacc.Bacc(
        "TRN2", target_bir_lowering=False, debug=False, num_devices=N_CORES
    )
    encT_d = nc.dram_tensor("encT", [B_LOC, NT, P, EO * 512], BF16, kind="ExternalInput").ap()
    encN_d = nc.dram_tensor("encN", [B_LOC, P, TO * E], BF16, kind="ExternalInput").ap()
    wenc_d = nc.dram_tensor("w_encT", [AO, P, EO * P], BF16, kind="ExternalInput").ap()
    wdec_d = nc.dram_tensor("w_decT", [2, P, DO * 512], BF16, kind="ExternalInput").ap()
    dech_d = nc.dram_tensor("dec_hT", [P, DO * B_LOC], BF16, kind="ExternalInput").ap()
    v_d = nc.dram_tensor("v_col", [P, AO], BF16, kind="ExternalInput").ap()
    wb_d = nc.dram_tensor("wb8", [P, AO], F32, kind="ExternalInput").ap()
    maskb_d = nc.dram_tensor("maskb", [B_LOC, TX], F32, kind="ExternalInput").ap()
    ctx_out = nc.dram_tensor("context", [B_LOC, E], F32, kind="ExternalOutput").ap()
    alpha_out = nc.dram_tensor("alpha", [B_LOC, TX], F32, kind="ExternalOutput").ap()

    with tile.TileContext(nc) as tc, ExitStack() as ctx:
        const = ctx.enter_context(tc.tile_pool(name="const", bufs=1))
        encT_pool = ctx.enter_context(tc.tile_pool(name="encTp", bufs=3))
        encN_pool = ctx.enter_context(tc.tile_pool(name="encNp", bufs=3))
        en_pool = ctx.enter_context(tc.tile_pool(name="energy", bufs=6))
        rowp = ctx.enter_context(tc.tile_pool(name="rows", bufs=2))
        small = ctx.enter_context(tc.tile_pool(name="small", bufs=2))
        ps_ep = ctx.enter_context(tc.tile_pool(name="ps_ep", bufs=3, space="PSUM"))
        ps_sc = ctx.enter_context(tc.tile_pool(name="ps_sc", bufs=2, space="PSUM"))
        ps_cx = ctx.enter_context(tc.tile_pool(name="ps_cx", bufs=2, space="PSUM"))
        ps_tiny = ctx.enter_context(tc.tile_pool(name="ps_tiny", bufs=1, space="PSUM"))

        wenc_sb = const.tile([P, AO, EO, P], BF16)
        wdec_sb = const.tile([P, 2, DO, 512], BF16)
        dech_sb = const.tile([P, DO, B_LOC], BF16)
        v_sb = const.tile([P, AO], BF16)
        wb_sb = const.tile([P, AO], F32)
        bias_sb = const.tile([P, AO, B_LOC], F32)
        ident4 = const.tile([B_LOC, B_LOC], F32)
        ident1 = const.tile([1, 1], F32)
        dumw = const.tile([B_LOC, B_LOC], F32)
        nc.vector.memset(dumw[:], 1.0)
        make_identity(nc, ident4[:])
        make_identity(nc, ident1[:])

        nc.gpsimd.dma_start(dech_sb[:].rearrange("p do b -> p (do b)"), dech_d[:])
        nc.gpsimd.dma_start(v_sb[:], v_d[:])
        nc.gpsimd.dma_start(wb_sb[:], wb_d[:])
        mask_rows = []
        for b in range(B_LOC):
            mr = small.tile([1, TX], F32, tag="mrow", bufs=B_LOC, name=f"mask{b}")
            nc.gpsimd.dma_start(mr[:], maskb_d[b : b + 1, :])
            mask_rows.append(mr)

        lanes3 = [nc.sync, nc.scalar, nc.gpsimd]
        lanes2 = [nc.sync, nc.gpsimd]
        li3 = [0]
        li2 = [0]

        def lane3():
            e = lanes3[li3[0] % 3]
            li3[0] += 1
            return e

        def lane2():
            e = lanes2[li2[0] % 2]
            li2[0] += 1
            return e

        def dma_chunks(dst2d, src2d, nchunks, lane_fn):
            n = src2d.shape[-1]
            step = n // nchunks
            for i in range(nchunks):
                lane_fn().dma_start(
                    dst2d[:, i * step : (i + 1) * step],
                    src2d[:, i * step : (i + 1) * step],
                )

        encT_tiles = {}
        encN_tiles = {}

        def alloc_encT(b):
            encT_tiles[b] = encT_pool.tile(
                [P, NT, EO, 512], BF16, tag="encT", name=f"encT{b}"
            )

        def alloc_encN(b):
            encN_tiles[b] = encN_pool.tile(
                [P, TO, E], BF16, tag="encN", name=f"encN{b}"
            )

        def encT_2d(b, nt):
            return encT_tiles[b][:, nt].rearrange("p eo j -> p (eo j)")

        def encN_2d(b):
            return encN_tiles[b][:].rearrange("p to e -> p (to e)")

        def wenc_2d(ao):
            return wenc_sb[:, ao].rearrange("p eo c -> p (eo c)")

        # ---- bulk loads in need order, chunked across the 3 queues ------
        alloc_encT(0)
        alloc_encT(1)
        alloc_encT(2)
        alloc_encN(0)
        alloc_encN(1)
        alloc_encN(2)
        wdec0_2d = wdec_sb[:, 0].rearrange("p do j -> p (do j)")
        wdec1_2d = wdec_sb[:, 1].rearrange("p do j -> p (do j)")
        dma_chunks(wdec0_2d, wdec_d[0], 2, lane3)
        dma_chunks(wenc_2d(0), wenc_d[0], 1, lane3)
        dma_chunks(encT_2d(0, 0), encT_d[0, 0], 2, lane3)
        dma_chunks(encT_2d(0, 1), encT_d[0, 1], 2, lane3)
        dma_chunks(wenc_2d(1), wenc_d[1], 1, lane3)
        dma_chunks(wenc_2d(2), wenc_d[2], 1, lane3)
        dma_chunks(wenc_2d(3), wenc_d[3], 1, lane3)
        dma_chunks(wdec1_2d, wdec_d[1], 2, lane3)
        dma_chunks(encT_2d(1, 0), encT_d[1, 0], 2, lane3)
        dma_chunks(encT_2d(1, 1), encT_d[1, 1], 2, lane3)
        for ao in range(4, 8):
            dma_chunks(wenc_2d(ao), wenc_d[ao], 1, lane3)
        dma_chunks(encN_2d(0), encN_d[0], 2, lane3)
        dma_chunks(encT_2d(2, 0), encT_d[2, 0], 1, lane3)
        dma_chunks(encT_2d(2, 1), encT_d[2, 1], 1, lane3)
        dma_chunks(encN_2d(1), encN_d[1], 2, lane3)
        dma_chunks(encN_2d(2), encN_d[2], 2, lane3)

        # ---- per-example state ------------------------------------------
        class Ex:
            pass

        exs = {}

        def get_ex(b):
            if b in exs:
                return exs[b]
            s = Ex()
            s.sc = [
                ps_sc.tile([1, 512], F32, tag="sc", name=f"sc{b}_{nt}")
                for nt in range(NT)
            ]
            s.msc = rowp.tile([1, TX], F32, tag="msc", name=f"msc{b}")
            s.expf = rowp.tile([1, TX], F32, tag="expf", name=f"expf{b}")
            s.s2 = small.tile([1, 2], F32, tag="s2", name=f"s2_{b}")
            s.expT_ps = ps_tiny.tile([P, TO], F32, tag="tiny", name=f"expTps{b}")
            s.expT = small.tile([P, TO], BF16, tag="expT", name=f"expT{b}")
            s.cx = [
                ps_cx.tile([1, 512], F32, tag="cx", name=f"cx{b}_{et}")
                for et in range(ET)
            ]
            s.alpha_row = rowp.tile([1, TX], F32, tag="arow", name=f"alpha{b}")
            s.ctx_row = rowp.tile([1, E], F32, tag="crow", name=f"ctx{b}")
            exs[b] = s
            return s

        # ---- compute blocks ---------------------------------------------
        dp_row = rowp.tile([B_LOC, A], F32, tag="dprow", bufs=1)

        def dec_half(h):
            dp = ps_ep.tile([P, 512], F32, tag="ep", name=f"dp{h}")
            for do in range(DO):
                nc.tensor.matmul(
                    dp[:B_LOC, :],
                    lhsT=dech_sb[:, do],
                    rhs=wdec_sb[:, h, do],
                    start=(do == 0),
                    stop=(do == DO - 1),
                )
            nc.vector.tensor_copy(dp_row[:, h * 512 : (h + 1) * 512], dp[:B_LOC, :])
            for ao in range(4 * h, 4 * h + 4):
                tp = ps_cx.tile([P, B_LOC], F32, tag="cx", name=f"tp{ao}")
                nc.tensor.transpose(tp[:], dp_row[:, ao * P : (ao + 1) * P], ident4[:])
                nc.vector.tensor_scalar_add(bias_sb[:, ao], tp[:], wb_sb[:, ao : ao + 1])

        def e_group(b, ao):
            # both t-halves' energy tiles for this a-chunk; each w_encT
            # chunk loaded once serves the nt0/nt1 matmuls back to back
            s = get_ex(b)
            ep0 = ps_ep.tile([P, 512], F32, tag="ep", name=f"ep{b}_{ao}_0")
            ep1 = ps_ep.tile([P, 512], F32, tag="ep", name=f"ep{b}_{ao}_1")
            for eo in range(EO):
                nc.tensor.matmul(
                    ep0[:], lhsT=wenc_sb[:, ao, eo], rhs=encT_tiles[b][:, 0, eo],
                    start=(eo == 0), stop=(eo == EO - 1),
                )
                nc.tensor.matmul(
                    ep1[:], lhsT=wenc_sb[:, ao, eo], rhs=encT_tiles[b][:, 1, eo],
                    start=(eo == 0), stop=(eo == EO - 1),
                )
            for nt, ep in ((0, ep0), (1, ep1)):
                en = en_pool.tile([P, 512], BF16, tag="energy", name=f"en{b}_{nt}_{ao}")
                nc.scalar.activation(
                    en[:], ep[:], AF.Tanh, bias=bias_sb[:, ao, b : b + 1]
                )
                nc.tensor.matmul(
                    s.sc[nt][:],
                    lhsT=v_sb[:, ao : ao + 1],
                    rhs=en[:],
                    start=(ao == 0),
                    stop=(ao == AO - 1),
                )

        def half_block(b, nt):
            s = get_ex(b)
            hs = slice(nt * 512, (nt + 1) * 512)
            nc.vector.tensor_add(s.msc[:, hs], s.sc[nt][:], mask_rows[b][:, hs])
            nc.scalar.activation(
                s.expf[:, hs], s.msc[:, hs], AF.Exp,
                accum_out=s.s2[:, nt : nt + 1],
            )
            for i in range(4):
                to = nt * 4 + i
                nc.tensor.transpose(
                    s.expT_ps[:, to : to + 1],
                    s.expf[:, to * P : (to + 1) * P],
                    ident1[:],
                )
            nc.vector.tensor_copy(
                s.expT[:, nt * 4 : nt * 4 + 4], s.expT_ps[:, nt * 4 : nt * 4 + 4]
            )
            for i in range(4):
                to = nt * 4 + i
                for et in range(ET):
                    nc.tensor.matmul(
                        s.cx[et][:],
                        lhsT=s.expT[:, to : to + 1],
                        rhs=encN_tiles[b][:, to, et * 512 : (et + 1) * 512],
                        start=(to == 0),
                        stop=(to == TO - 1),
                    )

        def fin(b):
            s = get_ex(b)
            ssum = small.tile([1, 1], F32, tag="ssum", name=f"ssum{b}")
            nc.vector.tensor_add(ssum[:], s.s2[:, 0:1], s.s2[:, 1:2])
            rsum = small.tile([1, 1], F32, tag="rsum", name=f"rsum{b}")
            nc.vector.reciprocal(rsum[:], ssum[:])
            nc.vector.tensor_scalar_mul(s.alpha_row[:], s.expf[:], rsum[:])
            nc.sync.dma_start(alpha_out[b : b + 1, :], s.alpha_row[:])
            for et in range(ET):
                nc.vector.tensor_scalar_mul(
                    s.ctx_row[:, et * 512 : (et + 1) * 512], s.cx[et][:], rsum[:]
                )
            nc.sync.dma_start(ctx_out[b : b + 1, :], s.ctx_row[:])

        def e_group_nt(b, ao, nt):
            # single-half energy group (startup only: runs as soon as one
            # encT half-slab has landed)
            s = get_ex(b)
            ep = ps_ep.tile([P, 512], F32, tag="ep", name=f"epn{b}_{ao}_{nt}")
            for eo in range(EO):
                nc.tensor.matmul(
                    ep[:], lhsT=wenc_sb[:, ao, eo], rhs=encT_tiles[b][:, nt, eo],
                    start=(eo == 0), stop=(eo == EO - 1),
                )
            en = en_pool.tile([P, 512], BF16, tag="energy", name=f"enn{b}_{nt}_{ao}")
            nc.scalar.activation(en[:], ep[:], AF.Tanh, bias=bias_sb[:, ao, b : b + 1])
            nc.tensor.matmul(
                s.sc[nt][:], lhsT=v_sb[:, ao : ao + 1], rhs=en[:],
                start=(ao == 0), stop=(ao == AO - 1),
            )

        # ---- software-pipelined program ---------------------------------
        # keep the PE clock (HAM) warm through the DMA-bound startup with
        # tiny dependency-free matmuls interleaved between the early blocks
        warm_ps = ps_tiny.tile([P, B_LOC], F32, tag="tiny", name="warm")

        def warm(n):
            for _ in range(n):
                nc.tensor.matmul(
                    warm_ps[:B_LOC, :B_LOC], lhsT=dumw[:], rhs=dumw[:],
                    start=True, stop=True,
                )

        warm(48)
        dec_half(0)
        warm(16)
        e_group_nt(0, 0, 0)
        warm(16)
        e_group_nt(0, 0, 1)
        warm(16)
        e_group_nt(0, 1, 0)
        warm(16)
        e_group_nt(0, 1, 1)
        warm(16)
        dec_half(1)
        warm(16)
        for ao in range(2, 8):
            e_group(0, ao)
            if ao <= 4:
                warm(12)

        for b in range(B_LOC):
            nb = b + 1
            if b == 1:
                alloc_encT(3)
                dma_chunks(encT_2d(3, 0), encT_d[3, 0], 1, lane2)
                dma_chunks(encT_2d(3, 1), encT_d[3, 1], 1, lane2)
                alloc_encN(3)
                dma_chunks(encN_2d(3), encN_d[3], 2, lane2)
            if nb < B_LOC:
                # cover this example's softmax tail with the next
                # example's first energy groups
                e_group(nb, 0)
                e_group(nb, 1)
            half_block(b, 0)
            half_block(b, 1)
            fin(b)
            if nb < B_LOC:
                for ao in range(2, 8):
                    e_group(nb, ao)

    nc.compile()
    return nc


_NC = None


def _get_nc():
    global _NC
    if _NC is None:
        _NC = build_nc()
    return _NC


def make_in_maps(dec_hidden, enc_outputs, mask, W_w, W_b, v_w):
    dec_hidden = np.asarray(dec_hidden, np.float32)
    enc_outputs = np.asarray(enc_outputs, np.float32)
    mask = np.asarray(mask)
    W_w = np.asarray(W_w, np.float32)
    W_b = np.asarray(W_b, np.float32)
    v_w = np.asarray(v_w, np.float32)

    enc16 = enc_outputs.astype(BF)                       # [B, TX, E]
    # encT[b, nt, p, eo*512+j] = enc[b, nt*512+j, eo*128+p]
    x = enc16.reshape(-1, NT, 512, EO, P)
    encT = np.ascontiguousarray(x.transpose(0, 1, 4, 3, 2)).reshape(
        -1, NT, P, EO * 512
    )
    # encN[b, p, to*1024+e] = enc[b, to*128+p, e]
    y = enc16.reshape(-1, TO, P, E)
    encN = np.ascontiguousarray(y.transpose(0, 2, 1, 3)).reshape(-1, P, TO * E)

    W_enc = W_w[:, D:].astype(BF)                        # [A, E]
    wencT = np.ascontiguousarray(
        W_enc.reshape(AO, P, EO, P).transpose(0, 3, 2, 1)
    ).reshape(AO, P, EO * P)
    W_dec = W_w[:, :D].astype(BF)                        # [A, D]
    wdecT = np.ascontiguousarray(
        W_dec.T.reshape(DO, P, 2, 512).transpose(2, 1, 0, 3)
    ).reshape(2, P, DO * 512)
    wb8 = np.ascontiguousarray(W_b.reshape(AO, P).T)     # [P, AO] f32
    v16 = np.ascontiguousarray(v_w[0].reshape(AO, P).T.astype(BF))
    maskb = (mask.astype(np.float32) - 1.0) * 50.0       # 0 kept / -50 masked

    in_maps = []
    for c in range(N_CORES):
        sl = slice(B_LOC * c, B_LOC * (c + 1))
        dech = np.ascontiguousarray(
            dec_hidden[sl].T.reshape(DO, P, B_LOC).transpose(1, 0, 2)
        ).reshape(P, DO * B_LOC).astype(BF)
        in_maps.append(
            {
                "encT": np.ascontiguousarray(encT[sl]),
                "encN": np.ascontiguousarray(encN[sl]),
                "w_encT": wencT,
                "w_decT": wdecT,
                "dec_hT": dech,
                "v_col": v16,
                "wb8": wb8,
                "maskb": np.ascontiguousarray(maskb[sl]),
            }
        )
    return in_maps


def kernel(dec_hidden, enc_outputs, mask, W_w, W_b, v_w):
    from concourse.bass_utils import run_bass_kernel_spmd

    assert enc_outputs.shape == (N_CORES * B_LOC, TX, E), enc_outputs.shape
    nc = _get_nc()
    in_maps = make_in_maps(dec_hidden, enc_outputs, mask, W_w, W_b, v_w)
    res = run_bass_kernel_spmd(nc, in_maps, list(range(N_CORES))).results
    context = np.concatenate([res[c]["context"] for c in range(N_CORES)], axis=0)
    alpha = np.concatenate([res[c]["alpha"] for c in range(N_CORES)], axis=0)
    return context, alpha


# revision 15
# speedup vs baseline: 1.0218x; 1.0218x over previous
"""Bahdanau attention kernel for Trainium2 (Bass/Tile), 8-core data-parallel.

Problem shapes: B=32, Tx=1024, enc_hid=dec_hid=attn=1024.

v8: bf16 + nt-paired weight sharing + HAM warmup + startup split.
  - All big matmul operands bf16 (validated: ctx rel ~2.8e-3, tol 2e-2).
  - Host pre-tiles every tensor so each DMA is a contiguous 2D slab.
  - Energy groups pair the two 512-wide t-halves: each w_encT chunk is
    loaded once and serves the nt0/nt1 matmuls back to back (measured
    ~46ns weight-switch penalty per matmul otherwise), and consecutive
    matmuls alternate PSUM banks.
  - exp via ACT accum_out => softmax sum free; alpha row->column via
    tiny PE transposes (no DRAM bounce).
  - PE stream software-pipelined: the next example's first two energy
    groups are emitted before the current example's softmax tail.

Math (per example b):
  dec_proj = W_dec @ dec_hidden[b]                 [attn]
  energy^T[a, t] = tanh(sum_e W_enc[a,e] enc[b,t,e] + dec_proj[a] + W_b[a])
  scores[t] = sum_a v[a] energy^T[a, t]
  alpha = softmax(scores + (mask-1)*50)
  context[e] = sum_t alpha[t] enc[b,t,e]
"""

from contextlib import ExitStack

import numpy as np
import ml_dtypes

import concourse.bass as bass
import concourse.tile as tile
from concourse import bacc, mybir
from concourse.masks import make_identity

F32 = mybir.dt.float32
BF16 = mybir.dt.bfloat16
AF = mybir.ActivationFunctionType
BF = ml_dtypes.bfloat16

P = 128
N_CORES = 8
B_LOC = 4            # examples per core
TX = 1024
E = 1024             # enc_hid
A = 1024             # attn
D = 1024             # dec_hid
EO = E // P
AO = A // P
TO = TX // P
DO = D // P
NT = 2               # 512-wide t-halves
ET = 2               # 512-wide e-halves


def build_nc():
    nc = bacc.Bacc(
        "TRN2", target_bir_lowering=False, debug=False, num_devices=N_CORES
    )
    encT_d = nc.dram_tensor("encT", [B_LOC, NT, P, EO * 512], BF16, kind="ExternalInput").ap()
    encN_d = nc.dram_tensor("encN", [B_LOC, P, TO * E], BF16, kind="ExternalInput").ap()
    wenc_d = nc.dram_tensor("w_encT", [AO, P, EO * P], BF16, kind="ExternalInput").ap()
    wdec_d = nc.dram_tensor("w_decT", [2, P, DO * 512], BF16, kind="ExternalInput").ap()
    dech_d = nc.dram_tensor("dec_hT", [P, DO * B_LOC], BF16, kind="ExternalInput").ap()
    v_d = nc.dram_tensor("v_col", [P, AO], BF16, kind="ExternalInput").ap()
    wb_d = nc.dram_tensor("wb8", [P, AO], F32, kind="ExternalInput").ap()
    maskb_d = nc.dram_tensor("maskb", [B_LOC, TX], F32, kind="ExternalInput").ap()
    ctx_out = nc.dram_tensor("context", [B_LOC, E], F32, kind="ExternalOutput").ap()
    alpha_out = nc.dram_tensor("alpha", [B_LOC, TX], F32, kind="ExternalOutput").ap()

    with tile.TileContext(nc) as tc, ExitStack() as ctx:
        const = ctx.enter_context(tc.tile_pool(name="const", bufs=1))
        encT_pool = ctx.enter_context(tc.tile_pool(name="encTp", bufs=3))
        encN_pool = ctx.enter_context(tc.tile_pool(name="encNp", bufs=3))
        en_pool = ctx.enter_context(tc.tile_pool(name="energy", bufs=6))
        rowp = ctx.enter_context(tc.tile_pool(name="rows", bufs=2))
        small = ctx.enter_context(tc.tile_pool(name="small", bufs=2))
        ps_ep = ctx.enter_context(tc.tile_pool(name="ps_ep", bufs=3, space="PSUM"))
        ps_sc = ctx.enter_context(tc.tile_pool(name="ps_sc", bufs=2, space="PSUM"))
        ps_cx = ctx.enter_context(tc.tile_pool(name="ps_cx", bufs=2, space="PSUM"))
        ps_tiny = ctx.enter_context(tc.tile_pool(name="ps_tiny", bufs=1, space="PSUM"))

        wenc_sb = const.tile([P, AO, EO, P], BF16)
        wdec_sb = const.tile([P, 2, DO, 512], BF16)
        dech_sb = const.tile([P, DO, B_LOC], BF16)
        v_sb = const.tile([P, AO], BF16)
        wb_sb = const.tile([P, AO], F32)
        bias_sb = const.tile([P, AO, B_LOC], F32)
        ident4 = const.tile([B_LOC, B_LOC], F32)
        ident1 = const.tile([1, 1], F32)
        make_identity(nc, ident4[:])
        make_identity(nc, ident1[:])

        nc.gpsimd.dma_start(dech_sb[:].rearrange("p do b -> p (do b)"), dech_d[:])
        nc.gpsimd.dma_start(v_sb[:], v_d[:])
        nc.gpsimd.dma_start(wb_sb[:], wb_d[:])
        mask_rows = []
        for b in range(B_LOC):
            mr = small.tile([1, TX], F32, tag="mrow", bufs=B_LOC, name=f"mask{b}")
            nc.gpsimd.dma_start(mr[:], maskb_d[b : b + 1, :])
            mask_rows.append(mr)

        lanes3 = [nc.sync, nc.scalar, nc.gpsimd]
        lanes2 = [nc.sync, nc.gpsimd]
        li3 = [0]
        li2 = [0]

        def lane3():
            e = lanes3[li3[0] % 3]
            li3[0] += 1
            return e

        def lane2():
            e = lanes2[li2[0] % 2]
            li2[0] += 1
            return e

        def dma_chunks(dst2d, src2d, nchunks, lane_fn):
            n = src2d.shape[-1]
            step = n // nchunks
            for i in range(nchunks):
                lane_fn().dma_start(
                    dst2d[:, i * step : (i + 1) * step],
                    src2d[:, i * step : (i + 1) * step],
                )

        encT_tiles = {}
        encN_tiles = {}

        def alloc_encT(b):
            encT_tiles[b] = encT_pool.tile(
                [P, NT, EO, 512], BF16, tag="encT", name=f"encT{b}"
            )

        def alloc_encN(b):
            encN_tiles[b] = encN_pool.tile(
                [P, TO, E], BF16, tag="encN", name=f"encN{b}"
            )

        def encT_2d(b, nt):
            return encT_tiles[b][:, nt].rearrange("p eo j -> p (eo j)")

        def encN_2d(b):
            return encN_tiles[b][:].rearrange("p to e -> p (to e)")

        def wenc_2d(ao):
            return wenc_sb[:, ao].rearrange("p eo c -> p (eo c)")

        # ---- bulk loads in need order, chunked across the 3 queues ------
        alloc_encT(0)
        alloc_encT(1)
        alloc_encT(2)
        alloc_encN(0)
        alloc_encN(1)
        alloc_encN(2)
        wdec0_2d = wdec_sb[:, 0].rearrange("p do j -> p (do j)")
        wdec1_2d = wdec_sb[:, 1].rearrange("p do j -> p (do j)")
        dma_chunks(wdec0_2d, wdec_d[0], 2, lane3)
        dma_chunks(wenc_2d(0), wenc_d[0], 1, lane3)
        dma_chunks(encT_2d(0, 0), encT_d[0, 0], 2, lane3)
        dma_chunks(encT_2d(0, 1), encT_d[0, 1], 2, lane3)
        dma_chunks(wenc_2d(1), wenc_d[1], 1, lane3)
        dma_chunks(wenc_2d(2), wenc_d[2], 1, lane3)
        dma_chunks(wenc_2d(3), wenc_d[3], 1, lane3)
        dma_chunks(wdec1_2d, wdec_d[1], 2, lane3)
        dma_chunks(encT_2d(1, 0), encT_d[1, 0], 2, lane3)
        dma_chunks(encT_2d(1, 1), encT_d[1, 1], 2, lane3)
        for ao in range(4, 8):
            dma_chunks(wenc_2d(ao), wenc_d[ao], 1, lane3)
        dma_chunks(encN_2d(0), encN_d[0], 2, lane3)
        dma_chunks(encT_2d(2, 0), encT_d[2, 0], 1, lane3)
        dma_chunks(encT_2d(2, 1), encT_d[2, 1], 1, lane3)
        dma_chunks(encN_2d(1), encN_d[1], 2, lane3)
        dma_chunks(encN_2d(2), encN_d[2], 2, lane3)

        # ---- per-example state ------------------------------------------
        class Ex:
            pass

        exs = {}

        def get_ex(b):
            if b in exs:
                return exs[b]
            s = Ex()
            s.sc = [
                ps_sc.tile([1, 512], F32, tag="sc", name=f"sc{b}_{nt}")
                for nt in range(NT)
            ]
            s.msc = rowp.tile([1, TX], F32, tag="msc", name=f"msc{b}")
            s.expf = rowp.tile([1, TX], F32, tag="expf", name=f"expf{b}")
            s.s2 = small.tile([1, 2], F32, tag="s2", name=f"s2_{b}")
            s.expT_ps = ps_tiny.tile([P, TO], F32, tag="tiny", name=f"expTps{b}")
            s.expT = small.tile([P, TO], BF16, tag="expT", name=f"expT{b}")
            s.cx = [
                ps_cx.tile([1, 512], F32, tag="cx", name=f"cx{b}_{et}")
                for et in range(ET)
            ]
            s.alpha_row = rowp.tile([1, TX], F32, tag="arow", name=f"alpha{b}")
            s.ctx_row = rowp.tile([1, E], F32, tag="crow", name=f"ctx{b}")
            exs[b] = s
            return s

        # ---- compute blocks ---------------------------------------------
        dp_row = rowp.tile([B_LOC, A], F32, tag="dprow", bufs=1)

        def dec_half(h):
            dp = ps_ep.tile([P, 512], F32, tag="ep", name=f"dp{h}")
            for do in range(DO):
                nc.tensor.matmul(
                    dp[:B_LOC, :],
                    lhsT=dech_sb[:, do],
                    rhs=wdec_sb[:, h, do],
                    start=(do == 0),
                    stop=(do == DO - 1),
                )
            nc.vector.tensor_copy(dp_row[:, h * 512 : (h + 1) * 512], dp[:B_LOC, :])
            for ao in range(4 * h, 4 * h + 4):
                tp = ps_tiny.tile([P, B_LOC], F32, tag="tiny", name=f"tp{ao}")
                nc.tensor.transpose(tp[:], dp_row[:, ao * P : (ao + 1) * P], ident4[:])
                nc.vector.tensor_scalar_add(bias_sb[:, ao], tp[:], wb_sb[:, ao : ao + 1])

        def e_group(b, ao):
            # both t-halves' energy tiles for this a-chunk; each w_encT
            # chunk loaded once serves the nt0/nt1 matmuls back to back
            s = get_ex(b)
            ep0 = ps_ep.tile([P, 512], F32, tag="ep", name=f"ep{b}_{ao}_0")
            ep1 = ps_ep.tile([P, 512], F32, tag="ep", name=f"ep{b}_{ao}_1")
            for eo in range(EO):
                nc.tensor.matmul(
                    ep0[:], lhsT=wenc_sb[:, ao, eo], rhs=encT_tiles[b][:, 0, eo],
                    start=(eo == 0), stop=(eo == EO - 1),
                )
                nc.tensor.matmul(
                    ep1[:], lhsT=wenc_sb[:, ao, eo], rhs=encT_tiles[b][:, 1, eo],
                    start=(eo == 0), stop=(eo == EO - 1),
                )
            for nt, ep in ((0, ep0), (1, ep1)):
                en = en_pool.tile([P, 512], BF16, tag="energy", name=f"en{b}_{nt}_{ao}")
                nc.scalar.activation(
                    en[:], ep[:], AF.Tanh, bias=bias_sb[:, ao, b : b + 1]
                )
                nc.tensor.matmul(
                    s.sc[nt][:],
                    lhsT=v_sb[:, ao : ao + 1],
                    rhs=en[:],
                    start=(ao == 0),
                    stop=(ao == AO - 1),
                )

        def half_block(b, nt):
            s = get_ex(b)
            hs = slice(nt * 512, (nt + 1) * 512)
            nc.vector.tensor_add(s.msc[:, hs], s.sc[nt][:], mask_rows[b][:, hs])
            nc.scalar.activation(
                s.expf[:, hs], s.msc[:, hs], AF.Exp,
                accum_out=s.s2[:, nt : nt + 1],
            )
            for i in range(4):
                to = nt * 4 + i
                nc.tensor.transpose(
                    s.expT_ps[:, to : to + 1],
                    s.expf[:, to * P : (to + 1) * P],
                    ident1[:],
                )
            nc.vector.tensor_copy(
                s.expT[:, nt * 4 : nt * 4 + 4], s.expT_ps[:, nt * 4 : nt * 4 + 4]
            )
            for i in range(4):
                to = nt * 4 + i
                for et in range(ET):
                    nc.tensor.matmul(
                        s.cx[et][:],
                        lhsT=s.expT[:, to : to + 1],
                        rhs=encN_tiles[b][:, to, et * 512 : (et + 1) * 512],
                        start=(to == 0),
                        stop=(to == TO - 1),
                    )

        def fin(b):
            s = get_ex(b)
            ssum = small.tile([1, 1], F32, tag="ssum", name=f"ssum{b}")
            nc.vector.tensor_add(ssum[:], s.s2[:, 0:1], s.s2[:, 1:2])
            rsum = small.tile([1, 1], F32, tag="rsum", name=f"rsum{b}")
            nc.vector.reciprocal(rsum[:], ssum[:])
            nc.vector.tensor_scalar_mul(s.alpha_row[:], s.expf[:], rsum[:])
            nc.sync.dma_start(alpha_out[b : b + 1, :], s.alpha_row[:])
            for et in range(ET):
                nc.vector.tensor_scalar_mul(
                    s.ctx_row[:, et * 512 : (et + 1) * 512], s.cx[et][:], rsum[:]
                )
            nc.sync.dma_start(ctx_out[b : b + 1, :], s.ctx_row[:])

        def e_group_nt(b, ao, nt):
            # single-half energy group (startup only: runs as soon as one
            # encT half-slab has landed)
            s = get_ex(b)
            ep = ps_ep.tile([P, 512], F32, tag="ep", name=f"epn{b}_{ao}_{nt}")
            for eo in range(EO):
                nc.tensor.matmul(
                    ep[:], lhsT=wenc_sb[:, ao, eo], rhs=encT_tiles[b][:, nt, eo],
                    start=(eo == 0), stop=(eo == EO - 1),
                )
            en = en_pool.tile([P, 512], BF16, tag="energy", name=f"enn{b}_{nt}_{ao}")
            nc.scalar.activation(en[:], ep[:], AF.Tanh, bias=bias_sb[:, ao, b : b + 1])
            nc.tensor.matmul(
                s.sc[nt][:], lhsT=v_sb[:, ao : ao + 1], rhs=en[:],
                start=(ao == 0), stop=(ao == AO - 1),
            )

        # ---- software-pipelined program ---------------------------------
        # warm the PE clock (HAM) with tiny dependency-free matmuls during
        # the DMA-bound startup so real matmuls start at full rate
        warm = ps_tiny.tile([P, B_LOC], F32, tag="tiny", name="warm")
        for _ in range(64):
            nc.tensor.matmul(
                warm[:B_LOC, :B_LOC], lhsT=ident4[:], rhs=ident4[:],
                start=True, stop=True,
            )
        dec_half(0)
        e_group_nt(0, 0, 0)
        e_group_nt(0, 0, 1)
        e_group_nt(0, 1, 0)
        e_group_nt(0, 1, 1)
        dec_half(1)
        for ao in range(2, 8):
            e_group(0, ao)

        for b in range(B_LOC):
            nb = b + 1
            if b == 1:
                alloc_encT(3)
                dma_chunks(encT_2d(3, 0), encT_d[3, 0], 1, lane2)
                dma_chunks(encT_2d(3, 1), encT_d[3, 1], 1, lane2)
                alloc_encN(3)
                dma_chunks(encN_2d(3), encN_d[3], 2, lane2)
            if nb < B_LOC:
                # cover this example's softmax tail with the next
                # example's first energy groups
                e_group(nb, 0)
                e_group(nb, 1)
            half_block(b, 0)
            half_block(b, 1)
            fin(b)
            if nb < B_LOC:
                for ao in range(2, 8):
                    e_group(nb, ao)

    nc.compile()
    return nc


_NC = None


def _get_nc():
    global _NC
    if _NC is None:
        _NC = build_nc()
    return _NC


def make_in_maps(dec_hidden, enc_outputs, mask, W_w, W_b, v_w):
    dec_hidden = np.asarray(dec_hidden, np.float32)
    enc_outputs = np.asarray(enc_outputs, np.float32)
    mask = np.asarray(mask)
    W_w = np.asarray(W_w, np.float32)
    W_b = np.asarray(W_b, np.float32)
    v_w = np.asarray(v_w, np.float32)

    enc16 = enc_outputs.astype(BF)                       # [B, TX, E]
    # encT[b, nt, p, eo*512+j] = enc[b, nt*512+j, eo*128+p]
    x = enc16.reshape(-1, NT, 512, EO, P)
    encT = np.ascontiguousarray(x.transpose(0, 1, 4, 3, 2)).reshape(
        -1, NT, P, EO * 512
    )
    # encN[b, p, to*1024+e] = enc[b, to*128+p, e]
    y = enc16.reshape(-1, TO, P, E)
    encN = np.ascontiguousarray(y.transpose(0, 2, 1, 3)).reshape(-1, P, TO * E)

    W_enc = W_w[:, D:].astype(BF)                        # [A, E]
    wencT = np.ascontiguousarray(
        W_enc.reshape(AO, P, EO, P).transpose(0, 3, 2, 1)
    ).reshape(AO, P, EO * P)
    W_dec = W_w[:, :D].astype(BF)                        # [A, D]
    wdecT = np.ascontiguousarray(
        W_dec.T.reshape(DO, P, 2, 512).transpose(2, 1, 0, 3)
    ).reshape(2, P, DO * 512)
    wb8 = np.ascontiguousarray(W_b.reshape(AO, P).T)     # [P, AO] f32
    v16 = np.ascontiguousarray(v_w[0].reshape(AO, P).T.astype(BF))
    maskb = (mask.astype(np.float32) - 1.0) * 50.0       # 0 kept / -50 masked

    in_maps = []
    for c in range(N_CORES):
        sl = slice(B_LOC * c, B_LOC * (c + 1))
        dech = np.ascontiguousarray(
            dec_hidden[sl].T.reshape(DO, P, B_LOC).transpose(1, 0, 2)
        ).reshape(P, DO * B_LOC).astype(BF)
        in_maps.append(
            {
                "encT": np.ascontiguousarray(encT[sl]),
                "encN": np.ascontiguousarray(encN[sl]),
                "w_encT": wencT,
                "w_decT": wdecT,
                "dec_hT": dech,
                "v_col": v16,
                "wb8": wb8,
                "maskb": np.ascontiguousarray(maskb[sl]),
            }
        )
    return in_maps


def kernel(dec_hidden, enc_outputs, mask, W_w, W_b, v_w):
    from concourse.bass_utils import run_bass_kernel_spmd

    assert enc_outputs.shape == (N_CORES * B_LOC, TX, E), enc_outputs.shape
    nc = _get_nc()
    in_maps = make_in_maps(dec_hidden, enc_outputs, mask, W_w, W_b, v_w)
    res = run_bass_kernel_spmd(nc, in_maps, list(range(N_CORES))).results
    context = np.concatenate([res[c]["context"] for c in range(N_CORES)], axis=0)
    alpha = np.concatenate([res[c]["alpha"] for c in range(N_CORES)], axis=0)
    return context, alpha


# revision 16
# speedup vs baseline: 1.2522x; 1.2254x over previous
"""Bahdanau attention kernel for Trainium2 (Bass/Tile), 8-core data-parallel.

Problem shapes: B=32, Tx=1024, enc_hid=dec_hid=attn=1024.

v11: v8 + deferred score-matmul bursts (single v load, no tanh stalls).
  - All big matmul operands bf16 (validated: ctx rel ~2.8e-3, tol 2e-2).
  - Host pre-tiles every tensor so each DMA is a contiguous 2D slab.
  - Energy groups pair the two 512-wide t-halves: each w_encT chunk is
    loaded once and serves the nt0/nt1 matmuls back to back (measured
    ~46ns weight-switch penalty per matmul otherwise), and consecutive
    matmuls alternate PSUM banks.
  - exp via ACT accum_out => softmax sum free; alpha row->column via
    tiny PE transposes (no DRAM bounce).
  - PE stream software-pipelined: the next example's first two energy
    groups are emitted before the current example's softmax tail.

Math (per example b):
  dec_proj = W_dec @ dec_hidden[b]                 [attn]
  energy^T[a, t] = tanh(sum_e W_enc[a,e] enc[b,t,e] + dec_proj[a] + W_b[a])
  scores[t] = sum_a v[a] energy^T[a, t]
  alpha = softmax(scores + (mask-1)*50)
  context[e] = sum_t alpha[t] enc[b,t,e]
"""

from contextlib import ExitStack

import numpy as np
import ml_dtypes

import concourse.bass as bass
import concourse.tile as tile
from concourse import bacc, mybir
from concourse.masks import make_identity

F32 = mybir.dt.float32
BF16 = mybir.dt.bfloat16
AF = mybir.ActivationFunctionType
BF = ml_dtypes.bfloat16

P = 128
N_CORES = 8
B_LOC = 4            # examples per core
TX = 1024
E = 1024             # enc_hid
A = 1024             # attn
D = 1024             # dec_hid
EO = E // P
AO = A // P
TO = TX // P
DO = D // P
NT = 2               # 512-wide t-halves
ET = 2               # 512-wide e-halves


def build_nc():
    nc = bacc.Bacc(
        "TRN2", target_bir_lowering=False, debug=False, num_devices=N_CORES
    )
    encT_d = nc.dram_tensor("encT", [B_LOC, NT, P, EO * 512], BF16, kind="ExternalInput").ap()
    encN_d = nc.dram_tensor("encN", [B_LOC, P, TO * E], BF16, kind="ExternalInput").ap()
    wenc_d = nc.dram_tensor("w_encT", [AO, P, EO * P], BF16, kind="ExternalInput").ap()
    wdec_d = nc.dram_tensor("w_decT", [2, P, DO * 512], BF16, kind="ExternalInput").ap()
    dech_d = nc.dram_tensor("dec_hT", [P, DO * B_LOC], BF16, kind="ExternalInput").ap()
    v_d = nc.dram_tensor("v_col", [P, AO], BF16, kind="ExternalInput").ap()
    wb_d = nc.dram_tensor("wb8", [P, AO], F32, kind="ExternalInput").ap()
    maskb_d = nc.dram_tensor("maskb", [B_LOC, TX], F32, kind="ExternalInput").ap()
    ctx_out = nc.dram_tensor("context", [B_LOC, E], F32, kind="ExternalOutput").ap()
    alpha_out = nc.dram_tensor("alpha", [B_LOC, TX], F32, kind="ExternalOutput").ap()

    with tile.TileContext(nc) as tc, ExitStack() as ctx:
        const = ctx.enter_context(tc.tile_pool(name="const", bufs=1))
        encT_pool = ctx.enter_context(tc.tile_pool(name="encTp", bufs=3))
        encN_pool = ctx.enter_context(tc.tile_pool(name="encNp", bufs=2))
        en_pool = ctx.enter_context(tc.tile_pool(name="energy", bufs=20))
        rowp = ctx.enter_context(tc.tile_pool(name="rows", bufs=2))
        small = ctx.enter_context(tc.tile_pool(name="small", bufs=2))
        ps_ep = ctx.enter_context(tc.tile_pool(name="ps_ep", bufs=3, space="PSUM"))
        ps_sc = ctx.enter_context(tc.tile_pool(name="ps_sc", bufs=2, space="PSUM"))
        ps_cx = ctx.enter_context(tc.tile_pool(name="ps_cx", bufs=2, space="PSUM"))
        ps_tiny = ctx.enter_context(tc.tile_pool(name="ps_tiny", bufs=1, space="PSUM"))

        wenc_sb = const.tile([P, AO, EO, P], BF16)
        wdec_sb = const.tile([P, 2, DO, 512], BF16)
        dech_sb = const.tile([P, DO, B_LOC], BF16)
        v_sb = const.tile([P, AO], BF16)
        wb_sb = const.tile([P, AO], F32)
        bias_sb = const.tile([P, AO, B_LOC], F32)
        ident4 = const.tile([B_LOC, B_LOC], F32)
        ident1 = const.tile([1, 1], F32)
        dumw = const.tile([B_LOC, B_LOC], F32)
        nc.vector.memset(dumw[:], 1.0)
        make_identity(nc, ident4[:])
        make_identity(nc, ident1[:])

        nc.gpsimd.dma_start(dech_sb[:].rearrange("p do b -> p (do b)"), dech_d[:])
        nc.gpsimd.dma_start(v_sb[:], v_d[:])
        nc.gpsimd.dma_start(wb_sb[:], wb_d[:])
        mask_rows = []
        for b in range(B_LOC):
            mr = small.tile([1, TX], F32, tag="mrow", bufs=B_LOC, name=f"mask{b}")
            nc.gpsimd.dma_start(mr[:], maskb_d[b : b + 1, :])
            mask_rows.append(mr)

        lanes3 = [nc.sync, nc.scalar, nc.gpsimd]
        lanes2 = [nc.sync, nc.gpsimd]
        li3 = [0]
        li2 = [0]

        def lane3():
            e = lanes3[li3[0] % 3]
            li3[0] += 1
            return e

        def lane2():
            e = lanes2[li2[0] % 2]
            li2[0] += 1
            return e

        def dma_chunks(dst2d, src2d, nchunks, lane_fn):
            n = src2d.shape[-1]
            step = n // nchunks
            for i in range(nchunks):
                lane_fn().dma_start(
                    dst2d[:, i * step : (i + 1) * step],
                    src2d[:, i * step : (i + 1) * step],
                )

        encT_tiles = {}
        encN_tiles = {}

        def alloc_encT(b):
            encT_tiles[b] = encT_pool.tile(
                [P, NT, EO, 512], BF16, tag="encT", name=f"encT{b}"
            )

        def alloc_encN(b):
            encN_tiles[b] = encN_pool.tile(
                [P, TO, E], BF16, tag="encN", name=f"encN{b}"
            )

        def encT_2d(b, nt):
            return encT_tiles[b][:, nt].rearrange("p eo j -> p (eo j)")

        def encN_2d(b):
            return encN_tiles[b][:].rearrange("p to e -> p (to e)")

        def wenc_2d(ao):
            return wenc_sb[:, ao].rearrange("p eo c -> p (eo c)")

        # ---- bulk loads in need order, chunked across the 3 queues ------
        alloc_encT(0)
        alloc_encT(1)
        alloc_encT(2)
        alloc_encN(0)
        alloc_encN(1)
        wdec0_2d = wdec_sb[:, 0].rearrange("p do j -> p (do j)")
        wdec1_2d = wdec_sb[:, 1].rearrange("p do j -> p (do j)")
        dma_chunks(wdec0_2d, wdec_d[0], 2, lane3)
        dma_chunks(wenc_2d(0), wenc_d[0], 1, lane3)
        dma_chunks(encT_2d(0, 0), encT_d[0, 0], 2, lane3)
        dma_chunks(encT_2d(0, 1), encT_d[0, 1], 2, lane3)
        dma_chunks(wenc_2d(1), wenc_d[1], 1, lane3)
        dma_chunks(wenc_2d(2), wenc_d[2], 1, lane3)
        dma_chunks(wenc_2d(3), wenc_d[3], 1, lane3)
        dma_chunks(wdec1_2d, wdec_d[1], 2, lane3)
        dma_chunks(encT_2d(1, 0), encT_d[1, 0], 2, lane3)
        dma_chunks(encT_2d(1, 1), encT_d[1, 1], 2, lane3)
        for ao in range(4, 8):
            dma_chunks(wenc_2d(ao), wenc_d[ao], 1, lane3)
        dma_chunks(encN_2d(0), encN_d[0], 2, lane3)
        dma_chunks(encT_2d(2, 0), encT_d[2, 0], 1, lane3)
        dma_chunks(encT_2d(2, 1), encT_d[2, 1], 1, lane3)
        dma_chunks(encN_2d(1), encN_d[1], 2, lane3)

        # ---- per-example state ------------------------------------------
        class Ex:
            pass

        exs = {}

        def get_ex(b):
            if b in exs:
                return exs[b]
            s = Ex()
            s.sc = [
                ps_sc.tile([1, 512], F32, tag="sc", name=f"sc{b}_{nt}")
                for nt in range(NT)
            ]
            s.en = {}
            s.msc = rowp.tile([1, TX], F32, tag="msc", name=f"msc{b}")
            s.expf = rowp.tile([1, TX], F32, tag="expf", name=f"expf{b}")
            s.s2 = small.tile([1, 2], F32, tag="s2", name=f"s2_{b}")
            s.expT_ps = ps_tiny.tile([P, TO], F32, tag="tiny", name=f"expTps{b}")
            s.expT = small.tile([P, TO], BF16, tag="expT", name=f"expT{b}")
            s.cx = [
                ps_cx.tile([1, 512], F32, tag="cx", name=f"cx{b}_{et}")
                for et in range(ET)
            ]
            s.alpha_row = rowp.tile([1, TX], F32, tag="arow", name=f"alpha{b}")
            s.ctx_row = rowp.tile([1, E], F32, tag="crow", name=f"ctx{b}")
            exs[b] = s
            return s

        # ---- compute blocks ---------------------------------------------
        dp_row = rowp.tile([B_LOC, A], F32, tag="dprow", bufs=1)

        def dec_half(h):
            dp = ps_ep.tile([P, 512], F32, tag="ep", name=f"dp{h}")
            for do in range(DO):
                nc.tensor.matmul(
                    dp[:B_LOC, :],
                    lhsT=dech_sb[:, do],
                    rhs=wdec_sb[:, h, do],
                    start=(do == 0),
                    stop=(do == DO - 1),
                )
            nc.vector.tensor_copy(dp_row[:, h * 512 : (h + 1) * 512], dp[:B_LOC, :])
            for ao in range(4 * h, 4 * h + 4):
                tp = ps_tiny.tile([P, B_LOC], F32, tag="tiny", name=f"tp{ao}")
                nc.tensor.transpose(tp[:], dp_row[:, ao * P : (ao + 1) * P], ident4[:])
                nc.vector.tensor_scalar_add(bias_sb[:, ao], tp[:], wb_sb[:, ao : ao + 1])

        def e_group(b, ao):
            # both t-halves' energy tiles for this a-chunk; each w_encT
            # chunk loaded once serves the nt0/nt1 matmuls back to back
            s = get_ex(b)
            ep0 = ps_ep.tile([P, 512], F32, tag="ep", name=f"ep{b}_{ao}_0")
            ep1 = ps_ep.tile([P, 512], F32, tag="ep", name=f"ep{b}_{ao}_1")
            for eo in range(EO):
                nc.tensor.matmul(
                    ep0[:], lhsT=wenc_sb[:, ao, eo], rhs=encT_tiles[b][:, 0, eo],
                    start=(eo == 0), stop=(eo == EO - 1),
                )
                nc.tensor.matmul(
                    ep1[:], lhsT=wenc_sb[:, ao, eo], rhs=encT_tiles[b][:, 1, eo],
                    start=(eo == 0), stop=(eo == EO - 1),
                )
            for nt, ep in ((0, ep0), (1, ep1)):
                en = en_pool.tile([P, 512], BF16, tag="energy", name=f"en{b}_{nt}_{ao}")
                nc.scalar.activation(
                    en[:], ep[:], AF.Tanh, bias=bias_sb[:, ao, b : b + 1]
                )
                s.en[(nt, ao)] = en

        def sc_burst(b):
            # all 16 score matmuls in one run: a single v weight load,
            # alternating PSUM banks, tanh dependencies long resolved
            s = get_ex(b)
            for ao in range(AO):
                for nt in range(NT):
                    nc.tensor.matmul(
                        s.sc[nt][:],
                        lhsT=v_sb[:, ao : ao + 1],
                        rhs=s.en[(nt, ao)][:],
                        start=(ao == 0),
                        stop=(ao == AO - 1),
                    )

        def half_block(b, nt):
            s = get_ex(b)
            hs = slice(nt * 512, (nt + 1) * 512)
            nc.vector.tensor_add(s.msc[:, hs], s.sc[nt][:], mask_rows[b][:, hs])
            nc.scalar.activation(
                s.expf[:, hs], s.msc[:, hs], AF.Exp,
                accum_out=s.s2[:, nt : nt + 1],
            )
            for i in range(4):
                to = nt * 4 + i
                nc.tensor.transpose(
                    s.expT_ps[:, to : to + 1],
                    s.expf[:, to * P : (to + 1) * P],
                    ident1[:],
                )
            nc.vector.tensor_copy(
                s.expT[:, nt * 4 : nt * 4 + 4], s.expT_ps[:, nt * 4 : nt * 4 + 4]
            )
            for i in range(4):
                to = nt * 4 + i
                for et in range(ET):
                    nc.tensor.matmul(
                        s.cx[et][:],
                        lhsT=s.expT[:, to : to + 1],
                        rhs=encN_tiles[b][:, to, et * 512 : (et + 1) * 512],
                        start=(to == 0),
                        stop=(to == TO - 1),
                    )

        def fin(b):
            s = get_ex(b)
            ssum = small.tile([1, 1], F32, tag="ssum", name=f"ssum{b}")
            nc.vector.tensor_add(ssum[:], s.s2[:, 0:1], s.s2[:, 1:2])
            rsum = small.tile([1, 1], F32, tag="rsum", name=f"rsum{b}")
            nc.vector.reciprocal(rsum[:], ssum[:])
            nc.vector.tensor_scalar_mul(s.alpha_row[:], s.expf[:], rsum[:])
            nc.sync.dma_start(alpha_out[b : b + 1, :], s.alpha_row[:])
            for et in range(ET):
                nc.vector.tensor_scalar_mul(
                    s.ctx_row[:, et * 512 : (et + 1) * 512], s.cx[et][:], rsum[:]
                )
            nc.sync.dma_start(ctx_out[b : b + 1, :], s.ctx_row[:])

        def e_group_nt(b, ao, nt):
            # single-half energy group (startup only: runs as soon as one
            # encT half-slab has landed)
            s = get_ex(b)
            ep = ps_ep.tile([P, 512], F32, tag="ep", name=f"epn{b}_{ao}_{nt}")
            for eo in range(EO):
                nc.tensor.matmul(
                    ep[:], lhsT=wenc_sb[:, ao, eo], rhs=encT_tiles[b][:, nt, eo],
                    start=(eo == 0), stop=(eo == EO - 1),
                )
            en = en_pool.tile([P, 512], BF16, tag="energy", name=f"enn{b}_{nt}_{ao}")
            nc.scalar.activation(en[:], ep[:], AF.Tanh, bias=bias_sb[:, ao, b : b + 1])
            s.en[(nt, ao)] = en

        # ---- software-pipelined program ---------------------------------
        # warm the PE clock (HAM) with tiny dependency-free matmuls during
        # the DMA-bound startup so real matmuls start at full rate
        warm = ps_tiny.tile([P, B_LOC], F32, tag="tiny", name="warm")
        for _ in range(64):
            nc.tensor.matmul(
                warm[:B_LOC, :B_LOC], lhsT=dumw[:], rhs=dumw[:],
                start=True, stop=True,
            )
        dec_half(0)
        e_group_nt(0, 0, 0)
        e_group_nt(0, 0, 1)
        e_group_nt(0, 1, 0)
        e_group_nt(0, 1, 1)
        dec_half(1)
        for ao in range(2, 8):
            e_group(0, ao)

        for b in range(B_LOC):
            nb = b + 1
            if b == 0:
                alloc_encN(2)
                dma_chunks(encN_2d(2), encN_d[2], 2, lane2)
            if b == 1:
                alloc_encT(3)
                dma_chunks(encT_2d(3, 0), encT_d[3, 0], 1, lane2)
                dma_chunks(encT_2d(3, 1), encT_d[3, 1], 1, lane2)
                alloc_encN(3)
                dma_chunks(encN_2d(3), encN_d[3], 2, lane2)
            sc_burst(b)
            if nb < B_LOC:
                # cover this example's softmax tail with the next
                # example's first energy groups
                e_group(nb, 0)
                e_group(nb, 1)
            half_block(b, 0)
            half_block(b, 1)
            fin(b)
            if nb < B_LOC:
                for ao in range(2, 8):
                    e_group(nb, ao)

    nc.compile()
    return nc


_NC = None


def _get_nc():
    global _NC
    if _NC is None:
        _NC = build_nc()
    return _NC


def make_in_maps(dec_hidden, enc_outputs, mask, W_w, W_b, v_w):
    dec_hidden = np.asarray(dec_hidden, np.float32)
    enc_outputs = np.asarray(enc_outputs, np.float32)
    mask = np.asarray(mask)
    W_w = np.asarray(W_w, np.float32)
    W_b = np.asarray(W_b, np.float32)
    v_w = np.asarray(v_w, np.float32)

    enc16 = enc_outputs.astype(BF)                       # [B, TX, E]
    # encT[b, nt, p, eo*512+j] = enc[b, nt*512+j, eo*128+p]
    x = enc16.reshape(-1, NT, 512, EO, P)
    encT = np.ascontiguousarray(x.transpose(0, 1, 4, 3, 2)).reshape(
        -1, NT, P, EO * 512
    )
    # encN[b, p, to*1024+e] = enc[b, to*128+p, e]
    y = enc16.reshape(-1, TO, P, E)
    encN = np.ascontiguousarray(y.transpose(0, 2, 1, 3)).reshape(-1, P, TO * E)

    W_enc = W_w[:, D:].astype(BF)                        # [A, E]
    wencT = np.ascontiguousarray(
        W_enc.reshape(AO, P, EO, P).transpose(0, 3, 2, 1)
    ).reshape(AO, P, EO * P)
    W_dec = W_w[:, :D].astype(BF)                        # [A, D]
    wdecT = np.ascontiguousarray(
        W_dec.T.reshape(DO, P, 2, 512).transpose(2, 1, 0, 3)
    ).reshape(2, P, DO * 512)
    wb8 = np.ascontiguousarray(W_b.reshape(AO, P).T)     # [P, AO] f32
    v16 = np.ascontiguousarray(v_w[0].reshape(AO, P).T.astype(BF))
    maskb = (mask.astype(np.float32) - 1.0) * 50.0       # 0 kept / -50 masked

    in_maps = []
    for c in range(N_CORES):
        sl = slice(B_LOC * c, B_LOC * (c + 1))
        dech = np.ascontiguousarray(
            dec_hidden[sl].T.reshape(DO, P, B_LOC).transpose(1, 0, 2)
        ).reshape(P, DO * B_LOC).astype(BF)
        in_maps.append(
            {
                "encT": np.ascontiguousarray(encT[sl]),
                "encN": np.ascontiguousarray(encN[sl]),
                "w_encT": wencT,
                "w_decT": wdecT,
                "dec_hT": dech,
                "v_col": v16,
                "wb8": wb8,
                "maskb": np.ascontiguousarray(maskb[sl]),
            }
        )
    return in_maps


def kernel(dec_hidden, enc_outputs, mask, W_w, W_b, v_w):
    from concourse.bass_utils import run_bass_kernel_spmd

    assert enc_outputs.shape == (N_CORES * B_LOC, TX, E), enc_outputs.shape
    nc = _get_nc()
    in_maps = make_in_maps(dec_hidden, enc_outputs, mask, W_w, W_b, v_w)
    res = run_bass_kernel_spmd(nc, in_maps, list(range(N_CORES))).results
    context = np.concatenate([res[c]["context"] for c in range(N_CORES)], axis=0)
    alpha = np.concatenate([res[c]["alpha"] for c in range(N_CORES)], axis=0)
    return context, alpha


# revision 17
# speedup vs baseline: 1.2670x; 1.0119x over previous
"""Bahdanau attention kernel for Trainium2 (Bass/Tile), 8-core data-parallel.

Problem shapes: B=32, Tx=1024, enc_hid=dec_hid=attn=1024.

v11: v8 + deferred score-matmul bursts (single v load, no tanh stalls).
  - All big matmul operands bf16 (validated: ctx rel ~2.8e-3, tol 2e-2).
  - Host pre-tiles every tensor so each DMA is a contiguous 2D slab.
  - Energy groups pair the two 512-wide t-halves: each w_encT chunk is
    loaded once and serves the nt0/nt1 matmuls back to back (measured
    ~46ns weight-switch penalty per matmul otherwise), and consecutive
    matmuls alternate PSUM banks.
  - exp via ACT accum_out => softmax sum free; alpha row->column via
    tiny PE transposes (no DRAM bounce).
  - PE stream software-pipelined: the next example's first two energy
    groups are emitted before the current example's softmax tail.

Math (per example b):
  dec_proj = W_dec @ dec_hidden[b]                 [attn]
  energy^T[a, t] = tanh(sum_e W_enc[a,e] enc[b,t,e] + dec_proj[a] + W_b[a])
  scores[t] = sum_a v[a] energy^T[a, t]
  alpha = softmax(scores + (mask-1)*50)
  context[e] = sum_t alpha[t] enc[b,t,e]
"""

from contextlib import ExitStack

import numpy as np
import ml_dtypes

import concourse.bass as bass
import concourse.tile as tile
from concourse import bacc, mybir
from concourse.masks import make_identity

F32 = mybir.dt.float32
BF16 = mybir.dt.bfloat16
AF = mybir.ActivationFunctionType
BF = ml_dtypes.bfloat16

P = 128
N_CORES = 8
B_LOC = 4            # examples per core
TX = 1024
E = 1024             # enc_hid
A = 1024             # attn
D = 1024             # dec_hid
EO = E // P
AO = A // P
TO = TX // P
DO = D // P
NT = 2               # 512-wide t-halves
ET = 2               # 512-wide e-halves


def build_nc():
    nc = bacc.Bacc(
        "TRN2", target_bir_lowering=False, debug=False, num_devices=N_CORES
    )
    encT_d = nc.dram_tensor("encT", [B_LOC, NT, P, EO * 512], BF16, kind="ExternalInput").ap()
    encN_d = nc.dram_tensor("encN", [B_LOC, P, TO * E], BF16, kind="ExternalInput").ap()
    wenc_d = nc.dram_tensor("w_encT", [AO, P, EO * P], BF16, kind="ExternalInput").ap()
    wdec_d = nc.dram_tensor("w_decT", [2, P, DO * 512], BF16, kind="ExternalInput").ap()
    dech_d = nc.dram_tensor("dec_hT", [P, DO * B_LOC], BF16, kind="ExternalInput").ap()
    v_d = nc.dram_tensor("v_col", [P, AO], BF16, kind="ExternalInput").ap()
    wb_d = nc.dram_tensor("wb8", [P, AO], F32, kind="ExternalInput").ap()
    maskb_d = nc.dram_tensor("maskb", [B_LOC, TX], F32, kind="ExternalInput").ap()
    ctx_out = nc.dram_tensor("context", [B_LOC, E], F32, kind="ExternalOutput").ap()
    alpha_out = nc.dram_tensor("alpha", [B_LOC, TX], F32, kind="ExternalOutput").ap()

    with tile.TileContext(nc) as tc, ExitStack() as ctx:
        const = ctx.enter_context(tc.tile_pool(name="const", bufs=1))
        encT_pool = ctx.enter_context(tc.tile_pool(name="encTp", bufs=3))
        encN_pool = ctx.enter_context(tc.tile_pool(name="encNp", bufs=2))
        en_pool = ctx.enter_context(tc.tile_pool(name="energy", bufs=20))
        rowp = ctx.enter_context(tc.tile_pool(name="rows", bufs=2))
        small = ctx.enter_context(tc.tile_pool(name="small", bufs=2))
        ps_ep = ctx.enter_context(tc.tile_pool(name="ps_ep", bufs=3, space="PSUM"))
        ps_sc = ctx.enter_context(tc.tile_pool(name="ps_sc", bufs=2, space="PSUM"))
        ps_cx = ctx.enter_context(tc.tile_pool(name="ps_cx", bufs=2, space="PSUM"))
        ps_tiny = ctx.enter_context(tc.tile_pool(name="ps_tiny", bufs=1, space="PSUM"))

        wenc_sb = const.tile([P, AO, EO, P], BF16)
        wdec_sb = const.tile([P, 2, DO, 512], BF16)
        dech_sb = const.tile([P, DO, B_LOC], BF16)
        v_sb = const.tile([P, AO], BF16)
        wb_sb = const.tile([P, AO], F32)
        bias_sb = const.tile([P, AO, B_LOC], F32)
        ident4 = const.tile([B_LOC, B_LOC], F32)
        ident1 = const.tile([1, 1], F32)
        dumw = const.tile([B_LOC, B_LOC], F32)
        nc.vector.memset(dumw[:], 1.0)
        make_identity(nc, ident4[:])
        make_identity(nc, ident1[:])

        nc.gpsimd.dma_start(dech_sb[:].rearrange("p do b -> p (do b)"), dech_d[:])
        nc.gpsimd.dma_start(v_sb[:], v_d[:])
        nc.gpsimd.dma_start(wb_sb[:], wb_d[:])
        mask_rows = []
        for b in range(B_LOC):
            mr = small.tile([1, TX], F32, tag="mrow", bufs=B_LOC, name=f"mask{b}")
            nc.gpsimd.dma_start(mr[:], maskb_d[b : b + 1, :])
            mask_rows.append(mr)

        lanes3 = [nc.sync, nc.scalar, nc.gpsimd]
        lanes2 = [nc.sync, nc.gpsimd]
        li3 = [0]
        li2 = [0]

        def lane3():
            e = lanes3[li3[0] % 3]
            li3[0] += 1
            return e

        def lane2():
            e = lanes2[li2[0] % 2]
            li2[0] += 1
            return e

        def dma_chunks(dst2d, src2d, nchunks, lane_fn):
            n = src2d.shape[-1]
            step = n // nchunks
            for i in range(nchunks):
                lane_fn().dma_start(
                    dst2d[:, i * step : (i + 1) * step],
                    src2d[:, i * step : (i + 1) * step],
                )

        encT_tiles = {}
        encN_tiles = {}

        def alloc_encT(b):
            encT_tiles[b] = encT_pool.tile(
                [P, NT, EO, 512], BF16, tag="encT", name=f"encT{b}"
            )

        def alloc_encN(b):
            encN_tiles[b] = encN_pool.tile(
                [P, TO, E], BF16, tag="encN", name=f"encN{b}"
            )

        def encT_2d(b, nt):
            return encT_tiles[b][:, nt].rearrange("p eo j -> p (eo j)")

        def encN_2d(b):
            return encN_tiles[b][:].rearrange("p to e -> p (to e)")

        def wenc_2d(ao):
            return wenc_sb[:, ao].rearrange("p eo c -> p (eo c)")

        # ---- bulk loads in need order, chunked across the 3 queues ------
        alloc_encT(0)
        alloc_encT(1)
        alloc_encT(2)
        alloc_encN(0)
        alloc_encN(1)
        wdec0_2d = wdec_sb[:, 0].rearrange("p do j -> p (do j)")
        wdec1_2d = wdec_sb[:, 1].rearrange("p do j -> p (do j)")
        dma_chunks(wdec0_2d, wdec_d[0], 2, lane3)
        dma_chunks(wenc_2d(0), wenc_d[0], 1, lane3)
        dma_chunks(encT_2d(0, 0), encT_d[0, 0], 2, lane3)
        dma_chunks(encT_2d(0, 1), encT_d[0, 1], 2, lane3)
        dma_chunks(wenc_2d(1), wenc_d[1], 1, lane3)
        dma_chunks(wenc_2d(2), wenc_d[2], 1, lane3)
        dma_chunks(wenc_2d(3), wenc_d[3], 1, lane3)
        dma_chunks(wdec1_2d, wdec_d[1], 2, lane3)
        dma_chunks(encT_2d(1, 0), encT_d[1, 0], 2, lane3)
        dma_chunks(encT_2d(1, 1), encT_d[1, 1], 2, lane3)
        for ao in range(4, 8):
            dma_chunks(wenc_2d(ao), wenc_d[ao], 1, lane3)
        dma_chunks(encN_2d(0), encN_d[0], 2, lane3)
        dma_chunks(encT_2d(2, 0), encT_d[2, 0], 1, lane3)
        dma_chunks(encT_2d(2, 1), encT_d[2, 1], 1, lane3)
        dma_chunks(encN_2d(1), encN_d[1], 2, lane3)

        # ---- per-example state ------------------------------------------
        class Ex:
            pass

        exs = {}

        def get_ex(b):
            if b in exs:
                return exs[b]
            s = Ex()
            s.sc = [
                ps_sc.tile([1, 512], F32, tag="sc", name=f"sc{b}_{nt}")
                for nt in range(NT)
            ]
            s.en = {}
            s.msc = rowp.tile([1, TX], F32, tag="msc", name=f"msc{b}")
            s.expf = rowp.tile([1, TX], F32, tag="expf", name=f"expf{b}")
            s.s2 = small.tile([1, 2], F32, tag="s2", name=f"s2_{b}")
            s.expT_ps = ps_tiny.tile([P, TO], F32, tag="tiny", name=f"expTps{b}")
            s.expT = small.tile([P, TO], BF16, tag="expT", name=f"expT{b}")
            s.cx = [
                ps_cx.tile([1, 512], F32, tag="cx", name=f"cx{b}_{et}")
                for et in range(ET)
            ]
            s.alpha_row = rowp.tile([1, TX], F32, tag="arow", name=f"alpha{b}")
            s.ctx_row = rowp.tile([1, E], F32, tag="crow", name=f"ctx{b}")
            exs[b] = s
            return s

        # ---- compute blocks ---------------------------------------------
        dp_row = rowp.tile([B_LOC, A], F32, tag="dprow", bufs=1)

        def dec_half(h):
            dp = ps_ep.tile([P, 512], F32, tag="ep", name=f"dp{h}")
            for do in range(DO):
                nc.tensor.matmul(
                    dp[:B_LOC, :],
                    lhsT=dech_sb[:, do],
                    rhs=wdec_sb[:, h, do],
                    start=(do == 0),
                    stop=(do == DO - 1),
                )
            nc.vector.tensor_copy(dp_row[:, h * 512 : (h + 1) * 512], dp[:B_LOC, :])
            for ao in range(4 * h, 4 * h + 4):
                tp = ps_tiny.tile([P, B_LOC], F32, tag="tiny", name=f"tp{ao}")
                nc.tensor.transpose(tp[:], dp_row[:, ao * P : (ao + 1) * P], ident4[:])
                nc.vector.tensor_scalar_add(bias_sb[:, ao], tp[:], wb_sb[:, ao : ao + 1])

        def e_group(b, ao):
            # both t-halves' energy tiles for this a-chunk; each w_encT
            # chunk loaded once serves the nt0/nt1 matmuls back to back
            s = get_ex(b)
            ep0 = ps_ep.tile([P, 512], F32, tag="ep", name=f"ep{b}_{ao}_0")
            ep1 = ps_ep.tile([P, 512], F32, tag="ep", name=f"ep{b}_{ao}_1")
            for eo in range(EO):
                nc.tensor.matmul(
                    ep0[:], lhsT=wenc_sb[:, ao, eo], rhs=encT_tiles[b][:, 0, eo],
                    start=(eo == 0), stop=(eo == EO - 1),
                )
                nc.tensor.matmul(
                    ep1[:], lhsT=wenc_sb[:, ao, eo], rhs=encT_tiles[b][:, 1, eo],
                    start=(eo == 0), stop=(eo == EO - 1),
                )
            for nt, ep in ((0, ep0), (1, ep1)):
                en = en_pool.tile([P, 512], BF16, tag="energy", name=f"en{b}_{nt}_{ao}")
                nc.scalar.activation(
                    en[:], ep[:], AF.Tanh, bias=bias_sb[:, ao, b : b + 1]
                )
                s.en[(nt, ao)] = en

        def sc_burst(b):
            # all 16 score matmuls in one run: a single v weight load,
            # alternating PSUM banks, tanh dependencies long resolved
            s = get_ex(b)
            for ao in range(AO):
                for nt in range(NT):
                    nc.tensor.matmul(
                        s.sc[nt][:],
                        lhsT=v_sb[:, ao : ao + 1],
                        rhs=s.en[(nt, ao)][:],
                        start=(ao == 0),
                        stop=(ao == AO - 1),
                    )

        def half_block(b, nt):
            s = get_ex(b)
            hs = slice(nt * 512, (nt + 1) * 512)
            nc.vector.tensor_add(s.msc[:, hs], s.sc[nt][:], mask_rows[b][:, hs])
            nc.scalar.activation(
                s.expf[:, hs], s.msc[:, hs], AF.Exp,
                accum_out=s.s2[:, nt : nt + 1],
            )
            for i in range(4):
                to = nt * 4 + i
                nc.tensor.transpose(
                    s.expT_ps[:, to : to + 1],
                    s.expf[:, to * P : (to + 1) * P],
                    ident1[:],
                )
            nc.vector.tensor_copy(
                s.expT[:, nt * 4 : nt * 4 + 4], s.expT_ps[:, nt * 4 : nt * 4 + 4]
            )
            for i in range(4):
                to = nt * 4 + i
                for et in range(ET):
                    nc.tensor.matmul(
                        s.cx[et][:],
                        lhsT=s.expT[:, to : to + 1],
                        rhs=encN_tiles[b][:, to, et * 512 : (et + 1) * 512],
                        start=(to == 0),
                        stop=(to == TO - 1),
                    )

        def fin(b):
            s = get_ex(b)
            ssum = small.tile([1, 1], F32, tag="ssum", name=f"ssum{b}")
            nc.vector.tensor_add(ssum[:], s.s2[:, 0:1], s.s2[:, 1:2])
            rsum = small.tile([1, 1], F32, tag="rsum", name=f"rsum{b}")
            nc.vector.reciprocal(rsum[:], ssum[:])
            nc.vector.tensor_scalar_mul(s.alpha_row[:], s.expf[:], rsum[:])
            nc.sync.dma_start(alpha_out[b : b + 1, :], s.alpha_row[:])
            for et in range(ET):
                nc.vector.tensor_scalar_mul(
                    s.ctx_row[:, et * 512 : (et + 1) * 512], s.cx[et][:], rsum[:]
                )
            nc.sync.dma_start(ctx_out[b : b + 1, :], s.ctx_row[:])

        def e_group_nt(b, ao, nt):
            # single-half energy group (startup only: runs as soon as one
            # encT half-slab has landed)
            s = get_ex(b)
            ep = ps_ep.tile([P, 512], F32, tag="ep", name=f"epn{b}_{ao}_{nt}")
            for eo in range(EO):
                nc.tensor.matmul(
                    ep[:], lhsT=wenc_sb[:, ao, eo], rhs=encT_tiles[b][:, nt, eo],
                    start=(eo == 0), stop=(eo == EO - 1),
                )
            en = en_pool.tile([P, 512], BF16, tag="energy", name=f"enn{b}_{nt}_{ao}")
            nc.scalar.activation(en[:], ep[:], AF.Tanh, bias=bias_sb[:, ao, b : b + 1])
            s.en[(nt, ao)] = en

        # ---- software-pipelined program ---------------------------------
        # warm the PE clock (HAM) with tiny dependency-free matmuls during
        # the DMA-bound startup so real matmuls start at full rate
        warm = ps_tiny.tile([P, B_LOC], F32, tag="tiny", name="warm")
        for _ in range(112):
            nc.tensor.matmul(
                warm[:B_LOC, :B_LOC], lhsT=dumw[:], rhs=dumw[:],
                start=True, stop=True,
            )
        dec_half(0)
        e_group_nt(0, 0, 0)
        e_group_nt(0, 0, 1)
        e_group_nt(0, 1, 0)
        e_group_nt(0, 1, 1)
        dec_half(1)
        for ao in range(2, 8):
            e_group(0, ao)

        for b in range(B_LOC):
            nb = b + 1
            if b == 0:
                alloc_encN(2)
                dma_chunks(encN_2d(2), encN_d[2], 2, lane2)
            if b == 1:
                alloc_encT(3)
                dma_chunks(encT_2d(3, 0), encT_d[3, 0], 1, lane2)
                dma_chunks(encT_2d(3, 1), encT_d[3, 1], 1, lane2)
                alloc_encN(3)
                dma_chunks(encN_2d(3), encN_d[3], 2, lane2)
            sc_burst(b)
            if nb < B_LOC:
                # cover this example's softmax tail with the next
                # example's first energy groups
                e_group(nb, 0)
                e_group(nb, 1)
            half_block(b, 0)
            half_block(b, 1)
            fin(b)
            if nb < B_LOC:
                for ao in range(2, 8):
                    e_group(nb, ao)

    nc.compile()
    return nc


_NC = None


def _get_nc():
    global _NC
    if _NC is None:
        _NC = build_nc()
    return _NC


def make_in_maps(dec_hidden, enc_outputs, mask, W_w, W_b, v_w):
    dec_hidden = np.asarray(dec_hidden, np.float32)
    enc_outputs = np.asarray(enc_outputs, np.float32)
    mask = np.asarray(mask)
    W_w = np.asarray(W_w, np.float32)
    W_b = np.asarray(W_b, np.float32)
    v_w = np.asarray(v_w, np.float32)

    enc16 = enc_outputs.astype(BF)                       # [B, TX, E]
    # encT[b, nt, p, eo*512+j] = enc[b, nt*512+j, eo*128+p]
    x = enc16.reshape(-1, NT, 512, EO, P)
    encT = np.ascontiguousarray(x.transpose(0, 1, 4, 3, 2)).reshape(
        -1, NT, P, EO * 512
    )
    # encN[b, p, to*1024+e] = enc[b, to*128+p, e]
    y = enc16.reshape(-1, TO, P, E)
    encN = np.ascontiguousarray(y.transpose(0, 2, 1, 3)).reshape(-1, P, TO * E)

    W_enc = W_w[:, D:].astype(BF)                        # [A, E]
    wencT = np.ascontiguousarray(
        W_enc.reshape(AO, P, EO, P).transpose(0, 3, 2, 1)
    ).reshape(AO, P, EO * P)
    W_dec = W_w[:, :D].astype(BF)                        # [A, D]
    wdecT = np.ascontiguousarray(
        W_dec.T.reshape(DO, P, 2, 512).transpose(2, 1, 0, 3)
    ).reshape(2, P, DO * 512)
    wb8 = np.ascontiguousarray(W_b.reshape(AO, P).T)     # [P, AO] f32
    v16 = np.ascontiguousarray(v_w[0].reshape(AO, P).T.astype(BF))
    maskb = (mask.astype(np.float32) - 1.0) * 50.0       # 0 kept / -50 masked

    in_maps = []
    for c in range(N_CORES):
        sl = slice(B_LOC * c, B_LOC * (c + 1))
        dech = np.ascontiguousarray(
            dec_hidden[sl].T.reshape(DO, P, B_LOC).transpose(1, 0, 2)
        ).reshape(P, DO * B_LOC).astype(BF)
        in_maps.append(
            {
                "encT": np.ascontiguousarray(encT[sl]),
                "encN": np.ascontiguousarray(encN[sl]),
                "w_encT": wencT,
                "w_decT": wdecT,
                "dec_hT": dech,
                "v_col": v16,
                "wb8": wb8,
                "maskb": np.ascontiguousarray(maskb[sl]),
            }
        )
    return in_maps


def kernel(dec_hidden, enc_outputs, mask, W_w, W_b, v_w):
    from concourse.bass_utils import run_bass_kernel_spmd

    assert enc_outputs.shape == (N_CORES * B_LOC, TX, E), enc_outputs.shape
    nc = _get_nc()
    in_maps = make_in_maps(dec_hidden, enc_outputs, mask, W_w, W_b, v_w)
    res = run_bass_kernel_spmd(nc, in_maps, list(range(N_CORES))).results
    context = np.concatenate([res[c]["context"] for c in range(N_CORES)], axis=0)
    alpha = np.concatenate([res[c]["alpha"] for c in range(N_CORES)], axis=0)
    return context, alpha
